# revision 1
# baseline (speedup 1.0000x reference)
"""Trainium2 Bass kernel: batched 1-D linear interpolation on a uniform grid.

out[b, j] = y[b, i_j] + w_j * (y[b, i_j + 1] - y[b, i_j])

where i_j / w_j depend only on x_new (known at kernel-build time), so they are
precomputed on the host and shipped as small constant inputs.  The column
gather runs on GPSIMD (ap_gather), the lerp on DVE/ACT, streaming on HWDGE DMA.

Sharding: pure data parallel over the batch axis across 8 NeuronCores
(y_points rows 16384 -> 8 x 2048); x_new-derived constants are replicated.
"""

import numpy as np

BATCH = 16384
NUM_POINTS = 2048
M = 4096
N_CORES = 8
ROWS_PER_CORE = BATCH // N_CORES  # 2048
P = 128
N_TILES = ROWS_PER_CORE // P  # 16

_NC_CACHE = {}


def _build_nc():
    import concourse.bacc as bacc
    import concourse.mybir as mybir
    from concourse.tile import TileContext

    f32 = mybir.dt.float32
    i16 = mybir.dt.int16

    nc = bacc.Bacc()
    y = nc.dram_tensor("y", [ROWS_PER_CORE, NUM_POINTS], f32, kind="ExternalInput")
    idx1 = nc.dram_tensor("idx1", [P, M // 16], i16, kind="ExternalInput")
    idx2 = nc.dram_tensor("idx2", [P, M // 16], i16, kind="ExternalInput")
    wrep = nc.dram_tensor("w", [P, M], f32, kind="ExternalInput")
    out = nc.dram_tensor("out", [ROWS_PER_CORE, M], f32, kind="ExternalOutput")

    with TileContext(nc) as tc:
        with (
            tc.tile_pool(name="const", bufs=1) as cp,
            tc.tile_pool(name="yin", bufs=2) as yp,
            tc.tile_pool(name="work", bufs=2) as wp,
            tc.tile_pool(name="outp", bufs=2) as op,
        ):
            idx1_t = cp.tile([P, M // 16], i16, tag="idx1")
            idx2_t = cp.tile([P, M // 16], i16, tag="idx2")
            w_t = cp.tile([P, M], f32, tag="w")
            nc.sync.dma_start(out=idx1_t[:], in_=idx1[:])
            nc.sync.dma_start(out=idx2_t[:], in_=idx2[:])
            nc.sync.dma_start(out=w_t[:], in_=wrep[:])

            for i in range(N_TILES):
                y_t = yp.tile([P, NUM_POINTS], f32, tag="y")
                nc.sync.dma_start(out=y_t[:], in_=y[i * P : (i + 1) * P, :])

                g1 = wp.tile([P, M], f32, tag="g1")
                g2 = wp.tile([P, M], f32, tag="g2")
                nc.gpsimd.ap_gather(
                    g1[:], y_t[:], idx1_t[:],
                    channels=P, num_elems=NUM_POINTS, d=1, num_idxs=M,
                )
                nc.gpsimd.ap_gather(
                    g2[:], y_t[:], idx2_t[:],
                    channels=P, num_elems=NUM_POINTS, d=1, num_idxs=M,
                )
                # g2 <- (g2 - g1) * w ; out <- g1 + g2
                nc.vector.tensor_sub(g2[:], g2[:], g1[:])
                nc.vector.tensor_mul(g2[:], g2[:], w_t[:])
                o_t = op.tile([P, M], f32, tag="o")
                nc.any.tensor_add(o_t[:], g1[:], g2[:])
                nc.sync.dma_start(out=out[i * P : (i + 1) * P, :], in_=o_t[:])

    nc.compile()
    return nc


def _get_nc():
    if "nc" not in _NC_CACHE:
        _NC_CACHE["nc"] = _build_nc()
    return _NC_CACHE["nc"]


def _host_precompute(x_new):
    """Replicate the reference's searchsorted/weight math with the same jax
    ops on the same backend, so boundary decisions and weight rounding match
    the reference bit-for-bit (the device searchsorted/divide are not IEEE-
    exact, so numpy does NOT reproduce them)."""
    import jax.numpy as jnp

    x_new_j = jnp.asarray(np.asarray(x_new, dtype=np.float32))
    x_points = jnp.linspace(0.0, 1.0, NUM_POINTS, dtype=x_new_j.dtype)
    idxs = jnp.searchsorted(x_points, x_new_j, side="right") - 1
    idxs = jnp.clip(idxs, 0, NUM_POINTS - 2)
    x1 = x_points[idxs]
    x2 = x_points[idxs + 1]
    w = (x_new_j - x1) / (x2 - x1)
    return np.asarray(idxs).astype(np.int64), np.asarray(w, dtype=np.float32)


def _wrap_idx(idxs):
    """ap_gather index layout: [128, M//16] int16, j stored at
    (partition j%16 within each 16-partition group, free slot j//16)."""
    base = idxs.astype(np.int16).reshape(M // 16, 16).T  # [16, M//16]
    return np.ascontiguousarray(np.tile(base, (P // 16, 1)))  # [128, M//16]


def _make_in_maps(y_points, x_new):
    idxs, w = _host_precompute(np.asarray(x_new))
    idx1_w = _wrap_idx(idxs)
    idx2_w = _wrap_idx(idxs + 1)
    w_rep = np.ascontiguousarray(np.broadcast_to(w[None, :], (P, M)))
    y_full = np.ascontiguousarray(np.asarray(y_points, dtype=np.float32))
    in_maps = []
    for c in range(N_CORES):
        in_maps.append({
            "y": y_full[c * ROWS_PER_CORE : (c + 1) * ROWS_PER_CORE],
            "idx1": idx1_w,
            "idx2": idx2_w,
            "w": w_rep,
        })
    return in_maps


def run(y_points, x_new, trace=False, **spmd_kwargs):
    """Run the Bass kernel; returns (output, BassKernelResults)."""
    from concourse.bass_utils import run_bass_kernel_spmd

    nc = _get_nc()
    in_maps = _make_in_maps(y_points, x_new)
    res = run_bass_kernel_spmd(
        nc, in_maps, list(range(N_CORES)), trace=trace, **spmd_kwargs
    )
    out = np.concatenate([r["out"] for r in res.results], axis=0)
    return out, res


def kernel(y_points, x_new):
    out, _ = run(y_points, x_new)
    return out



# revision 2
# speedup vs baseline: 28.6520x; 28.6520x over previous
"""Trainium2 Bass kernel: batched 1-D linear interpolation on a uniform grid.

out[b, j] = (1 - w_j) * y[b, i_j] + w_j * y[b, i_j + 1]

Reformulated as a matmul  out = y @ G  with G[i_j, j] = 1 - w_j and
G[i_j + 1, j] = w_j (2 nonzeros per column, known on the host from x_new).
Queries are sorted by bin index i_j so that each chunk of <=128 sorted
queries touches a window of <=128 consecutive grid points; the chunk is then
a single 128x128x128 TensorE matmul:

    psum[rows128, q128] = yT_window[pts128, rows128].T @ G_chunk[pts128, q128]

y is shipped pre-transposed (points-major) and cast to bf16 on the host, so
each window is one contiguous 512 KiB DMA.  PSUM results are cast to bf16 on
the way to SBUF (DVE/ACT alternating) and streamed out in 1 MiB DMAs.  The
host un-permutes the sorted output columns during the unshard.

Sharding: pure data parallel over the batch axis across 8 NeuronCores
(y_points rows 16384 -> 8 x 2048); x_new-derived constants are replicated.
"""

import numpy as np

BATCH = 16384
NUM_POINTS = 2048
M = 4096
N_CORES = 8
ROWS_PER_CORE = BATCH // N_CORES  # 2048
P = 128
N_TILES = ROWS_PER_CORE // P  # 16
CH = 128  # queries per chunk (= G columns per matmul)

_NC_CACHE = {}
_PLAN_CACHE = {}


def _host_precompute(x_new):
    """Replicate the reference's searchsorted/weight math with the same jax
    ops on the same backend, so boundary decisions match the reference."""
    import jax.numpy as jnp

    x_new_j = jnp.asarray(np.asarray(x_new, dtype=np.float32))
    x_points = jnp.linspace(0.0, 1.0, NUM_POINTS, dtype=x_new_j.dtype)
    idxs = jnp.searchsorted(x_points, x_new_j, side="right") - 1
    idxs = jnp.clip(idxs, 0, NUM_POINTS - 2)
    x1 = x_points[idxs]
    x2 = x_points[idxs + 1]
    w = (x_new_j - x1) / (x2 - x1)
    return np.asarray(idxs).astype(np.int64), np.asarray(w, dtype=np.float32)


def _make_plan(x_new):
    """Sort queries by bin index and chunk greedily: each chunk holds up to
    CH sorted queries whose grid window [i_min, i_max+1] fits in 128 points.
    Returns (chunks, order) where chunks = [(p0, qlo, qhi)] over sorted
    positions and order = argsort of the queries."""
    idxs, w = _host_precompute(x_new)
    order = np.argsort(idxs, kind="stable")
    si = idxs[order]

    chunks = []
    qlo = 0
    while qlo < M:
        qhi = min(qlo + CH, M)
        # shrink until window fits: need points [i_min, i_max + 1], 128 wide
        while si[qhi - 1] - si[qlo] > P - 2:
            qhi -= 1
        p0 = int(min(si[qlo], NUM_POINTS - P))
        chunks.append((p0, qlo, qhi))
        qlo = qhi
    return chunks, order, idxs, w


def _build_nc(chunks):
    import concourse.bacc as bacc
    import concourse.mybir as mybir
    from concourse.tile import TileContext

    f32 = mybir.dt.float32
    bf16 = mybir.dt.bfloat16
    nch = len(chunks)

    nc = bacc.Bacc()
    yt = nc.dram_tensor("yt", [NUM_POINTS, ROWS_PER_CORE], bf16, kind="ExternalInput")
    g = nc.dram_tensor("g", [P, nch * CH], bf16, kind="ExternalInput")
    out = nc.dram_tensor("out", [ROWS_PER_CORE, nch * CH], bf16, kind="ExternalOutput")

    with TileContext(nc) as tc:
        with (
            tc.tile_pool(name="const", bufs=1) as cp,
            tc.tile_pool(name="psum", bufs=8, space="PSUM") as pp,
            tc.tile_pool(name="outp", bufs=3) as op,
        ):
            g_t = cp.tile([P, nch * CH], bf16, tag="g")
            nc.sync.dma_start(out=g_t[:], in_=g[:])

            win = []
            for c, (p0, _, _) in enumerate(chunks):
                w_t = cp.tile([P, ROWS_PER_CORE], bf16, tag=f"win{c}")
                nc.sync.dma_start(out=w_t[:], in_=yt[p0 : p0 + P, :])
                win.append(w_t)

            for r in range(N_TILES):
                o_t = op.tile([P, nch * CH], bf16, tag="o")
                for c in range(nch):
                    ps = pp.tile([P, CH], f32, tag="ps")
                    nc.tensor.matmul(
                        ps[:],
                        win[c][:, r * P : (r + 1) * P],
                        g_t[:, c * CH : (c + 1) * CH],
                        start=True,
                        stop=True,
                    )
                    dst = o_t[:, c * CH : (c + 1) * CH]
                    # split the PSUM->SBUF cast copies across DVE and ACT
                    if c % 2 == 0:
                        nc.vector.tensor_copy(out=dst, in_=ps[:])
                    else:
                        nc.scalar.copy(dst, ps[:])
                nc.sync.dma_start(out=out[r * P : (r + 1) * P, :], in_=o_t[:])

    nc.compile()
    return nc


def _get_plan_and_nc(x_new):
    import ml_dtypes

    key = np.asarray(x_new, dtype=np.float32).tobytes()
    if key not in _PLAN_CACHE:
        chunks, order, idxs, w = _make_plan(x_new)
        nch = len(chunks)
        # G: [128 pts-in-window, nch*CH sorted queries], bf16
        gmat = np.zeros((P, nch * CH), dtype=np.float32)
        si = idxs[order]
        sw = w[order]
        for c, (p0, qlo, qhi) in enumerate(chunks):
            cols = c * CH + np.arange(qhi - qlo)
            gmat[si[qlo:qhi] - p0, cols] = 1.0 - sw[qlo:qhi]
            gmat[si[qlo:qhi] + 1 - p0, cols] = sw[qlo:qhi]
        gmat = gmat.astype(ml_dtypes.bfloat16)
        # natural output column for each device column slot (-1 = padding)
        cols_nat = np.full(nch * CH, -1, dtype=np.int64)
        for c, (p0, qlo, qhi) in enumerate(chunks):
            cols_nat[c * CH : c * CH + (qhi - qlo)] = order[qlo:qhi]
        _PLAN_CACHE[key] = (chunks, gmat, cols_nat)
    chunks, gmat, cols_nat = _PLAN_CACHE[key]

    nc_key = (len(chunks), tuple(p0 for p0, _, _ in chunks))
    if nc_key not in _NC_CACHE:
        _NC_CACHE[nc_key] = _build_nc(chunks)
    return chunks, gmat, cols_nat, _NC_CACHE[nc_key]


def run(y_points, x_new, trace=False, **spmd_kwargs):
    """Run the Bass kernel; returns (output, BassKernelResults)."""
    import ml_dtypes
    from concourse.bass_utils import run_bass_kernel_spmd

    chunks, gmat, cols_nat, nc = _get_plan_and_nc(x_new)

    y16 = np.asarray(y_points, dtype=np.float32).astype(ml_dtypes.bfloat16)
    in_maps = []
    for c in range(N_CORES):
        ytc = np.ascontiguousarray(y16[c * ROWS_PER_CORE : (c + 1) * ROWS_PER_CORE].T)
        in_maps.append({"yt": ytc, "g": gmat})

    res = run_bass_kernel_spmd(
        nc, in_maps, list(range(N_CORES)), trace=trace, **spmd_kwargs
    )

    valid = cols_nat >= 0
    dst_cols = cols_nat[valid]
    out_full = np.empty((BATCH, M), dtype=np.float32)
    for c in range(N_CORES):
        o = res.results[c]["out"]
        if not valid.all():
            o = o[:, valid]
        out_full[c * ROWS_PER_CORE : (c + 1) * ROWS_PER_CORE, dst_cols] = o.astype(
            np.float32
        )
    return out_full, res


def kernel(y_points, x_new):
    out, _ = run(y_points, x_new)
    return out


# revision 4
# speedup vs baseline: 36.2023x; 1.2635x over previous
"""Trainium2 Bass kernel: batched 1-D linear interpolation on a uniform grid.

out[b, j] = (1 - w_j) * y[b, i_j] + w_j * y[b, i_j + 1]

Reformulated as a matmul  out = y @ G  with G[i_j, j] = 1 - w_j and
G[i_j + 1, j] = w_j (2 nonzeros per column, known on the host from x_new).
Queries are sorted by bin index i_j so that each chunk of <=128 sorted
queries touches a window of <=128 consecutive grid points; the chunk is then
a single 128x128x128 TensorE matmul:

    psum[rows128, q128] = yT_window[pts128, rows128].T @ G_chunk[pts128, q128]

y is shipped pre-transposed (points-major) and cast to bf16 on the host, so
each window is one contiguous 512 KiB DMA.  PSUM results are cast to bf16 on
the way to SBUF (DVE/ACT alternating) and streamed out in 1 MiB DMAs.  The
host un-permutes the sorted output columns during the unshard.

Sharding: pure data parallel over the batch axis across 8 NeuronCores
(y_points rows 16384 -> 8 x 2048); x_new-derived constants are replicated.
"""

import numpy as np

BATCH = 16384
NUM_POINTS = 2048
M = 4096
N_CORES = 8
ROWS_PER_CORE = BATCH // N_CORES  # 2048
P = 128
N_TILES = ROWS_PER_CORE // P  # 16
CH = 128  # queries per chunk (= G columns per matmul)

_NC_CACHE = {}
_PLAN_CACHE = {}


def _host_precompute(x_new):
    """Replicate the reference's searchsorted/weight math with the same jax
    ops on the same backend, so boundary decisions match the reference."""
    import jax.numpy as jnp

    x_new_j = jnp.asarray(np.asarray(x_new, dtype=np.float32))
    x_points = jnp.linspace(0.0, 1.0, NUM_POINTS, dtype=x_new_j.dtype)
    idxs = jnp.searchsorted(x_points, x_new_j, side="right") - 1
    idxs = jnp.clip(idxs, 0, NUM_POINTS - 2)
    x1 = x_points[idxs]
    x2 = x_points[idxs + 1]
    w = (x_new_j - x1) / (x2 - x1)
    return np.asarray(idxs).astype(np.int64), np.asarray(w, dtype=np.float32)


def _make_plan(x_new):
    """Sort queries by bin index and chunk greedily: each chunk holds up to
    CH sorted queries whose grid window [i_min, i_max+1] fits in 128 points.
    Returns (chunks, order) where chunks = [(p0, qlo, qhi)] over sorted
    positions and order = argsort of the queries."""
    idxs, w = _host_precompute(x_new)
    order = np.argsort(idxs, kind="stable")
    si = idxs[order]

    chunks = []
    qlo = 0
    while qlo < M:
        qhi = min(qlo + CH, M)
        # shrink until window fits: need points [i_min, i_max + 1], 128 wide
        while si[qhi - 1] - si[qlo] > P - 2:
            qhi -= 1
        p0 = int(min(si[qlo], NUM_POINTS - P))
        chunks.append((p0, qlo, qhi))
        qlo = qhi
    return chunks, order, idxs, w


def _build_nc(chunks):
    import concourse.bacc as bacc
    import concourse.mybir as mybir
    from concourse.tile import TileContext

    f32 = mybir.dt.float32
    bf16 = mybir.dt.bfloat16
    nch = len(chunks)

    nc = bacc.Bacc()
    yt = nc.dram_tensor("yt", [NUM_POINTS, ROWS_PER_CORE], bf16, kind="ExternalInput")
    g = nc.dram_tensor("g", [P, nch * CH], bf16, kind="ExternalInput")
    out = nc.dram_tensor("out", [ROWS_PER_CORE, nch * CH], bf16, kind="ExternalOutput")

    BANK = 512  # one full PSUM bank (fp32); 4 chunk-matmuls per bank
    MM_PER_BANK = BANK // CH

    with TileContext(nc) as tc:
        with (
            tc.tile_pool(name="const", bufs=1) as cp,
            tc.tile_pool(name="psum", bufs=8, space="PSUM") as pp,
            tc.tile_pool(name="outp", bufs=4) as op,
        ):
            g_t = cp.tile([P, nch * CH], bf16, tag="g")
            win = [
                cp.tile([P, ROWS_PER_CORE], bf16, tag=f"win{c}", name=f"win{c}")
                for c in range(nch)
            ]
            # first matmul needs win0 + g cols 0:512 — issue those first,
            # split g so the first slice lands quickly
            nc.sync.dma_start(out=win[0][:], in_=yt[chunks[0][0] : chunks[0][0] + P, :])
            gq = (nch * CH) // 4
            for s in range(4):
                nc.sync.dma_start(
                    out=g_t[:, s * gq : (s + 1) * gq], in_=g[:, s * gq : (s + 1) * gq]
                )
            for c in range(1, nch):
                p0 = chunks[c][0]
                nc.sync.dma_start(out=win[c][:], in_=yt[p0 : p0 + P, :])

            nbank = (nch + MM_PER_BANK - 1) // MM_PER_BANK
            k = 0
            for r in range(N_TILES):
                o_t = op.tile([P, nch * CH], bf16, tag="o")
                for b in range(nbank):
                    clo = b * MM_PER_BANK
                    chi = min(clo + MM_PER_BANK, nch)
                    ps = pp.tile([P, BANK], f32, tag="ps")
                    for c in range(clo, chi):
                        nc.tensor.matmul(
                            ps[:, (c - clo) * CH : (c - clo + 1) * CH],
                            win[c][:, r * P : (r + 1) * P],
                            g_t[:, c * CH : (c + 1) * CH],
                            start=True,
                            stop=True,
                        )
                    dst = o_t[:, clo * CH : chi * CH]
                    src = ps[:, : (chi - clo) * CH]
                    # split the PSUM->SBUF cast copies across DVE and ACT (5:3)
                    if k % 8 < 5:
                        nc.vector.tensor_copy(out=dst, in_=src)
                    else:
                        nc.scalar.copy(dst, src)
                    k += 1
                # output DMAs on the second HWDGE ring (ACT) so they don't
                # serialize behind the input window loads on the sync ring
                nc.scalar.dma_start(out=out[r * P : (r + 1) * P, :], in_=o_t[:])

    nc.compile()
    return nc


def _get_plan_and_nc(x_new):
    import ml_dtypes

    key = np.asarray(x_new, dtype=np.float32).tobytes()
    if key not in _PLAN_CACHE:
        chunks, order, idxs, w = _make_plan(x_new)
        nch = len(chunks)
        # G: [128 pts-in-window, nch*CH sorted queries], bf16
        gmat = np.zeros((P, nch * CH), dtype=np.float32)
        si = idxs[order]
        sw = w[order]
        for c, (p0, qlo, qhi) in enumerate(chunks):
            cols = c * CH + np.arange(qhi - qlo)
            gmat[si[qlo:qhi] - p0, cols] = 1.0 - sw[qlo:qhi]
            gmat[si[qlo:qhi] + 1 - p0, cols] = sw[qlo:qhi]
        gmat = gmat.astype(ml_dtypes.bfloat16)
        # natural output column for each device column slot (-1 = padding)
        cols_nat = np.full(nch * CH, -1, dtype=np.int64)
        for c, (p0, qlo, qhi) in enumerate(chunks):
            cols_nat[c * CH : c * CH + (qhi - qlo)] = order[qlo:qhi]
        _PLAN_CACHE[key] = (chunks, gmat, cols_nat)
    chunks, gmat, cols_nat = _PLAN_CACHE[key]

    nc_key = (len(chunks), tuple(p0 for p0, _, _ in chunks))
    if nc_key not in _NC_CACHE:
        _NC_CACHE[nc_key] = _build_nc(chunks)
    return chunks, gmat, cols_nat, _NC_CACHE[nc_key]


def run(y_points, x_new, trace=False, **spmd_kwargs):
    """Run the Bass kernel; returns (output, BassKernelResults)."""
    import ml_dtypes
    from concourse.bass_utils import run_bass_kernel_spmd

    chunks, gmat, cols_nat, nc = _get_plan_and_nc(x_new)

    y16 = np.asarray(y_points, dtype=np.float32).astype(ml_dtypes.bfloat16)
    in_maps = []
    for c in range(N_CORES):
        ytc = np.ascontiguousarray(y16[c * ROWS_PER_CORE : (c + 1) * ROWS_PER_CORE].T)
        in_maps.append({"yt": ytc, "g": gmat})

    res = run_bass_kernel_spmd(
        nc, in_maps, list(range(N_CORES)), trace=trace, **spmd_kwargs
    )

    valid = cols_nat >= 0
    dst_cols = cols_nat[valid]
    out_full = np.empty((BATCH, M), dtype=np.float32)
    for c in range(N_CORES):
        o = res.results[c]["out"]
        if not valid.all():
            o = o[:, valid]
        out_full[c * ROWS_PER_CORE : (c + 1) * ROWS_PER_CORE, dst_cols] = o.astype(
            np.float32
        )
    return out_full, res


def kernel(y_points, x_new):
    out, _ = run(y_points, x_new)
    return out
